# revision 7
# baseline (speedup 1.0000x reference)
"""Trainium2 Bass kernel for block-causal (chunked) multi-head attention.

Computes, for x:[2,2048,1024], Wqkv:[3072,1024], Wout:[1024,1024]:
    qkv = x @ Wqkv.T ; per-head scaled scores; block-causal mask
    (causal OR same 64-chunk == full attention to all chunks <= own chunk);
    softmax; out = attn @ v ; y = out @ Wout.T

Sharding over 8 NeuronCores: data-parallel over batch (2) x tensor-parallel
over heads (16 heads -> 4 per core).  Each core projects q/k/v for its 4
heads, runs attention, and computes a partial output projection against its
256 columns of Wout; the host sums the 4 partials per batch element.

On-chip layout avoids all transposes: the host hands each core
  xT     [1024, 2048]  (x[b] transposed)
  wqkT   [1024, 512]   (Wqkv rows for its 4 heads' q,k -> transposed)
  wvT    [1024, 256]   (v rows transposed)
  woutT  [256, 1024]   (Wout columns for its head-slice, transposed)
Scores are computed transposed (S^T[tk, tq]) so that the attention matmul
P^T -> (attn @ V) needs no transposes, and the softmax denominator comes
for free from a ones-column appended to V.  The block-causal mask is
realized structurally: masked-out key blocks are simply never computed, and
the diagonal blocks use rectangular sub-views (chunk granularity 64).

Engines execute their instruction streams in order, so the emission is a
software pipeline over the 4 query tiles: the TensorE stream for the
(ScalarE-paced) attention of tile t is interleaved with "filler" matmul
chains -- the q/k/v projections of tile t+1 and the output projection of
tile t-1 -- keeping the PE busy through every exp dependency stall.
"""

import sys

if "/opt/trn_rl_repo" not in sys.path:
    sys.path.insert(0, "/opt/trn_rl_repo")

from collections import deque

import numpy as np

import concourse.bass as bass  # noqa: F401  (registers types)
import concourse.mybir as mybir
import concourse.tile as tile
from concourse import bacc
from concourse.bass_utils import run_bass_kernel_spmd

F32 = mybir.dt.float32
F32R = mybir.dt.float32r
EXP = mybir.ActivationFunctionType.Exp

B = 2
T = 2048
DIM = 1024
N_HEADS = 16
HD = 64
CHUNK = 64
H_PER_CORE = 4  # 16 heads / (8 cores / 2 batches)
QT = 512  # query tile (free dim of S^T matmuls)
KB = 128  # key block (contraction block of AV matmuls)
N_QT = T // QT  # 4
N_KB = T // KB  # 16
N_DIMB = DIM // 128  # 8 contraction blocks for the projections
SCALE = 1.0 / np.sqrt(HD)

_CACHED_NC = None


def _emit(nc, tc, xT, wqkT, wvT, woT, y):
    po = tc.tile_pool  # shorthand

    with (
        po(name="persist", bufs=1) as pp,
        po(name="s_ps", bufs=2, space="PSUM") as sps,  # [128,1024] score slots
        po(name="mm_ps", bufs=2, space="PSUM") as mmps,  # [128,512] proj/y slots
        po(name="ot_ps", bufs=2, space="PSUM") as otps,  # [65,512] outT slots
        po(name="pbuf", bufs=3) as ppool,  # exp(S^T) tiles
        po(name="osbuf", bufs=2) as ospool,  # assembled normalized outT
        po(name="scbuf", bufs=4) as scpool,  # normalize scratch
        po(name="rbuf", bufs=4) as rpool,  # reciprocal denominators
        po(name="ybuf", bufs=2) as ypool,
    ):
        # ---- persistent SBUF tensors (chunked to keep deps fine-grained) ----
        xt = [
            [pp.tile([128, QT], F32R, tag=f"xt{k}_{c}", name=f"xt{k}_{c}") for c in range(N_QT)]
            for k in range(N_DIMB)
        ]
        wqk = [pp.tile([128, 512], F32R, tag=f"wqk{k}", name=f"wqk{k}") for k in range(N_DIMB)]
        wv = [pp.tile([128, 256], F32R, tag=f"wv{k}", name=f"wv{k}") for k in range(N_DIMB)]
        wo = [pp.tile([128, DIM], F32R, tag=f"wo{d}", name=f"wo{d}") for d in range(2)]
        # q/k head-dim-major: partition block hp holds heads (2hp, 2hp+1)
        qt = [
            [pp.tile([128, QT], F32R, tag=f"qt{i}_{c}", name=f"qt{i}_{c}") for c in range(N_QT)]
            for i in range(2)
        ]
        kt = [
            [pp.tile([128, QT], F32R, tag=f"kt{i}_{c}", name=f"kt{i}_{c}") for c in range(N_QT)]
            for i in range(2)
        ]
        # v (token-major) + ones column, per key block: [128, 4 heads, 65]
        vh = [
            pp.tile([128, H_PER_CORE, HD + 1], F32R, tag=f"vh{b}", name=f"vh{b}")
            for b in range(N_KB)
        ]
        # ones row for the K=1 denominator-broadcast matmuls (row 64 used)
        ones = pp.tile([128, 64], F32R, tag="ones", name="ones")
        nc.vector.memset(ones[:].bitcast(F32), 1.0)

        # ---- input DMAs: kb-major so the kb=0..7 chains fill in order; the
        # xT columns arrive chunk-by-chunk so tile 0's projections start early
        for kb in range(N_DIMB):
            nc.sync.dma_start(wqk[kb][:], wqkT[kb * 128 : (kb + 1) * 128, :])
            nc.sync.dma_start(wv[kb][:], wvT[kb * 128 : (kb + 1) * 128, :])
            nc.sync.dma_start(xt[kb][0][:], xT[kb * 128 : (kb + 1) * 128, 0:QT])
        for ct in range(1, N_QT):
            cs = slice(ct * QT, (ct + 1) * QT)
            for kb in range(N_DIMB):
                nc.sync.dma_start(xt[kb][ct][:], xT[kb * 128 : (kb + 1) * 128, cs])
        for db in range(2):
            nc.sync.dma_start(wo[db][:], woT[db * 128 : (db + 1) * 128, :])

        def qk_chain(tt, ob):  # ob 0,1 -> q pair blocks; 2,3 -> k pair blocks
            ps = mmps.tile([128, 512], F32, tag="mm512", name=f"qk_ps{tt}_{ob}")
            for kb in range(N_DIMB):
                nc.tensor.matmul(
                    ps[:],
                    wqk[kb][:, ob * 128 : (ob + 1) * 128],
                    xt[kb][tt][:],
                    start=(kb == 0),
                    stop=(kb == N_DIMB - 1),
                )
            dest = (qt if ob < 2 else kt)[ob % 2][tt]
            nc.vector.tensor_copy(dest[:], ps[:])

        def v_chain(tb):
            ps = mmps.tile([128, 256], F32, tag="mm512", name=f"v_ps{tb}")
            for kb in range(N_DIMB):
                nc.tensor.matmul(
                    ps[:],
                    xt[kb][tb // 4][:, (tb % 4) * KB : (tb % 4 + 1) * KB],
                    wv[kb][:],
                    start=(kb == 0),
                    stop=(kb == N_DIMB - 1),
                )
            nc.vector.tensor_copy(vh[tb][:, :, 0:HD], ps[:])
            nc.vector.memset(vh[tb][:, :, HD : HD + 1].bitcast(F32), 1.0)

        def proj_pieces(tt):
            for ob in range(4):
                yield lambda ob=ob: qk_chain(tt, ob)
            for tb in range(4 * tt, 4 * tt + 4):
                yield lambda tb=tb: v_chain(tb)

        def y_pieces(tt, os_pair):
            for t4 in range(4):
                trows = slice(t4 * 128, (t4 + 1) * 128)
                ysb = ypool.tile([128, DIM], F32, tag="ysb", name=f"ysb{tt}_{t4}")
                for jb in range(2):

                    def piece(t4=t4, jb=jb, ysb=ysb, trows=trows):
                        yps = mmps.tile(
                            [128, 512], F32, tag="mm512", name=f"y_ps{tt}_{t4}_{jb}"
                        )
                        for db in range(2):
                            nc.tensor.matmul(
                                yps[:],
                                os_pair[db][:, trows],
                                wo[db][:, jb * 512 : (jb + 1) * 512],
                                start=(db == 0),
                                stop=(db == 1),
                            )
                        nc.vector.tensor_copy(ysb[:, jb * 512 : (jb + 1) * 512], yps[:])
                        if jb == 1:
                            nc.sync.dma_start(
                                y[tt * QT + t4 * 128 : tt * QT + (t4 + 1) * 128, :],
                                ysb[:],
                            )

                    yield piece

        def attend(tt, os_pair, fillers):
            nb = 4 * (tt + 1)  # allowed key blocks for this query tile
            n_steps = 2 * nb
            step = 0
            done_fill = 0
            n_fill = len(fillers)

            def fill():
                nonlocal done_fill
                want = (step + 1) * n_fill // n_steps
                while done_fill < want and fillers:
                    fillers.popleft()()
                    done_fill += 1

            for hp in range(2):  # head pair (2hp, 2hp+1)
                ot = [
                    otps.tile([HD + 1, QT], F32, tag="ot", name=f"ot{tt}_{hp}_{i}")
                    for i in range(2)
                ]

                def s_mm(b):
                    """S^T for key block b, both heads, into one 2-bank tile."""
                    diag = b - 4 * tt
                    d = diag * 128 if diag >= 0 else 0
                    s = sps.tile([128, 2 * QT], F32, tag="s2", name=f"s{tt}_{hp}_{b}")
                    for i in range(2):
                        rows = slice(i * 64, i * 64 + 64)
                        nc.tensor.matmul(
                            s[:, i * QT + d : (i + 1) * QT],
                            kt[hp][b // 4][rows, (b % 4) * KB : (b % 4 + 1) * KB],
                            qt[hp][tt][rows, d:QT],
                            start=True,
                            stop=True,
                        )
                    return s

                s_tiles = {0: s_mm(0)}
                for b in range(nb):
                    if b + 1 < nb:
                        s_tiles[b + 1] = s_mm(b + 1)
                    diag = b - 4 * tt
                    d = diag * 128 if diag >= 0 else 0
                    s = s_tiles.pop(b)
                    p = ppool.tile([128, 2 * QT], F32R, tag="p", name=f"p{tt}_{hp}_{b}")
                    if diag < 0:
                        nc.scalar.activation(p[:], s[:], EXP, scale=SCALE)
                    else:
                        # per head: rows 0-63 attend cols >= d, rows 64-127
                        # cols >= d+64 (64-chunk block-causal inside the tile)
                        s2 = s[:].rearrange("p (h c) -> p h c", h=2)
                        p2 = p[:].rearrange("p (h c) -> p h c", h=2)
                        nc.vector.memset(p2[64:128, :, d : d + 64].bitcast(F32), 0.0)
                        nc.scalar.activation(
                            p2[0:64, :, d:QT], s2[0:64, :, d:QT], EXP, scale=SCALE
                        )
                        if d + 64 < QT:
                            nc.scalar.activation(
                                p2[64:128, :, d + 64 : QT],
                                s2[64:128, :, d + 64 : QT],
                                EXP,
                                scale=SCALE,
                            )
                    for i in range(2):
                        nc.tensor.matmul(
                            ot[i][:, d:QT],
                            vh[b][:, 2 * hp + i, :],
                            p[:, i * QT + d : (i + 1) * QT],
                            start=(b == 0),
                            stop=(b == nb - 1),
                        )
                    fill()
                    step += 1

                # normalize: os_pair[hp][i*64:(i+1)*64] = ot[i][0:64] / ot[i][64]
                for i in range(2):
                    dn = rpool.tile([65, QT], F32R, tag="dn", name=f"dn{tt}_{hp}_{i}")
                    nc.vector.tensor_copy(dn[64:65, :], ot[i][64:65, :])
                    # broadcast the denominator row across 64 partitions via a
                    # K=1 ones matmul (gpsimd partition_broadcast mis-reads
                    # partition-64 sources on HW)
                    rbp = mmps.tile(
                        [128, 512], F32, tag="mm512", name=f"rbp{tt}_{hp}_{i}"
                    )
                    nc.tensor.matmul(
                        rbp[0:64, :], ones[64:65, :], dn[64:65, :],
                        start=True, stop=True,
                    )
                    rb = rpool.tile([64, QT], F32, tag="rb", name=f"rb{tt}_{hp}_{i}")
                    nc.vector.reciprocal(rb[:], rbp[0:64, :])
                    sc = scpool.tile([64, QT], F32R, tag="sc", name=f"sc{tt}_{hp}_{i}")
                    nc.vector.tensor_mul(sc[:], ot[i][0:64, :], rb[:])
                    # partition-shifting SBUF->SBUF copy via DMA
                    nc.sync.dma_start(os_pair[hp][i * 64 : (i + 1) * 64, :], sc[:])

            while fillers:
                fillers.popleft()()

        # ---- the pipeline ----
        for piece in proj_pieces(0):
            piece()
        prev_y = None
        for tt in range(N_QT):
            os_pair = [
                ospool.tile([128, QT], F32R, tag=f"os{i}", name=f"os{i}_{tt}")
                for i in range(2)
            ]
            fillers = deque()
            a = deque(proj_pieces(tt + 1)) if tt + 1 < N_QT else deque()
            b = deque(prev_y) if prev_y is not None else deque()
            while a or b:
                if a:
                    fillers.append(a.popleft())
                if b:
                    fillers.append(b.popleft())
            attend(tt, os_pair, fillers)
            prev_y = list(y_pieces(tt, os_pair))
        for piece in prev_y:
            piece()


def build():
    global _CACHED_NC
    if _CACHED_NC is not None:
        return _CACHED_NC
    nc = bacc.Bacc(
        "TRN2", target_bir_lowering=False, debug=False, enable_asserts=False
    )
    xT = nc.dram_tensor("xT", [DIM, T], F32R, kind="ExternalInput").ap()
    wqkT = nc.dram_tensor("wqkT", [DIM, 512], F32R, kind="ExternalInput").ap()
    wvT = nc.dram_tensor("wvT", [DIM, 256], F32R, kind="ExternalInput").ap()
    woT = nc.dram_tensor("woutT", [256, DIM], F32R, kind="ExternalInput").ap()
    y = nc.dram_tensor("y", [T, DIM], F32, kind="ExternalOutput").ap()
    with tile.TileContext(nc) as tc:
        _emit(nc, tc, xT, wqkT, wvT, woT, y)
    nc.compile()
    _CACHED_NC = nc
    return nc


def make_in_maps(x, Wqkv, Wout):
    """Host-side sharding: core c = (batch c//4, head-group c%4)."""
    in_maps = []
    for c in range(8):
        b, hg = divmod(c, 4)
        hs = hg * H_PER_CORE
        r0, r1 = hs * HD, (hs + H_PER_CORE) * HD
        qrows = Wqkv[r0:r1]
        krows = Wqkv[DIM + r0 : DIM + r1]
        vrows = Wqkv[2 * DIM + r0 : 2 * DIM + r1]
        in_maps.append(
            {
                "xT": np.ascontiguousarray(x[b].T),
                "wqkT": np.ascontiguousarray(np.concatenate([qrows, krows], 0).T),
                "wvT": np.ascontiguousarray(vrows.T),
                "woutT": np.ascontiguousarray(Wout[:, r0:r1].T),
            }
        )
    return in_maps


def kernel(x, Wqkv, Wout):
    x = np.asarray(x, dtype=np.float32)
    Wqkv = np.asarray(Wqkv, dtype=np.float32)
    Wout = np.asarray(Wout, dtype=np.float32)
    nc = build()
    in_maps = make_in_maps(x, Wqkv, Wout)
    res = run_bass_kernel_spmd(nc, in_maps, core_ids=list(range(8)))
    out = np.zeros((B, T, DIM), np.float32)
    for c in range(8):
        out[c // 4] += res.results[c]["y"]
    return out


# revision 8
# speedup vs baseline: 4.6193x; 4.6193x over previous
"""Trainium2 Bass kernel for block-causal (chunked) multi-head attention.

Computes, for x:[2,2048,1024], Wqkv:[3072,1024], Wout:[1024,1024]:
    qkv = x @ Wqkv.T ; per-head scaled scores; block-causal mask
    (causal OR same 64-chunk == full attention to all chunks <= own chunk);
    softmax; out = attn @ v ; y = out @ Wout.T

Sharding over 8 NeuronCores: data-parallel over batch (2) x tensor-parallel
over heads (16 heads -> 4 per core).  Each core projects q/k/v for its 4
heads, runs attention, and computes a partial output projection against its
256 columns of Wout; the host sums the 4 partials per batch element.

On-chip layout avoids all transposes: the host hands each core
  xT     [1024, 2048]  (x[b] transposed)
  wqkT   [1024, 512]   (Wqkv rows for its 4 heads' q,k -> transposed)
  wvT    [1024, 256]   (v rows transposed)
  woutT  [256, 1024]   (Wout columns for its head-slice, transposed)
Scores are computed transposed (S^T[tk, tq]) so that the attention matmul
P^T -> (attn @ V) needs no transposes, and the softmax denominator comes
for free from a ones-column appended to V.  The block-causal mask is
realized structurally: masked-out key blocks are simply never computed, and
the diagonal blocks use rectangular sub-views (chunk granularity 64).

Engines execute their instruction streams in order, so the emission is a
software pipeline over the 4 query tiles: the TensorE stream for the
(ScalarE-paced) attention of tile t is interleaved with "filler" matmul
chains -- the q/k/v projections of tile t+1 and the output projection of
tile t-1 -- keeping the PE busy through every exp dependency stall.
"""

import sys

if "/opt/trn_rl_repo" not in sys.path:
    sys.path.insert(0, "/opt/trn_rl_repo")

from collections import deque

import numpy as np

import concourse.bass as bass  # noqa: F401  (registers types)
import concourse.mybir as mybir
import concourse.tile as tile
from concourse import bacc
from concourse.bass_utils import run_bass_kernel_spmd

F32 = mybir.dt.float32
F32R = mybir.dt.float32r
EXP = mybir.ActivationFunctionType.Exp

B = 2
T = 2048
DIM = 1024
N_HEADS = 16
HD = 64
CHUNK = 64
H_PER_CORE = 4  # 16 heads / (8 cores / 2 batches)
QT = 512  # query tile (free dim of S^T matmuls)
KB = 128  # key block (contraction block of AV matmuls)
N_QT = T // QT  # 4
N_KB = T // KB  # 16
N_DIMB = DIM // 128  # 8 contraction blocks for the projections
SCALE = 1.0 / np.sqrt(HD)

_CACHED_NC = None


def _emit(nc, tc, xT, wqkT, wvT, woT, y):
    po = tc.tile_pool  # shorthand

    with (
        po(name="persist", bufs=1) as pp,
        po(name="s_ps", bufs=2, space="PSUM") as sps,  # [128,1024] score slots
        po(name="mm_ps", bufs=2, space="PSUM") as mmps,  # [128,512] proj/y slots
        po(name="ot_ps", bufs=2, space="PSUM") as otps,  # [65,512] outT slots
        po(name="pbuf", bufs=4) as ppool,  # exp(S^T) tiles
        po(name="osbuf", bufs=2) as ospool,  # assembled normalized outT
        po(name="scbuf", bufs=4) as scpool,  # normalize scratch
        po(name="rbuf", bufs=4) as rpool,  # reciprocal denominators
        po(name="ybuf", bufs=3) as ypool,
    ):
        # ---- persistent SBUF tensors (chunked to keep deps fine-grained) ----
        xt = [
            [pp.tile([128, QT], F32R, tag=f"xt{k}_{c}", name=f"xt{k}_{c}") for c in range(N_QT)]
            for k in range(N_DIMB)
        ]
        wqk = [pp.tile([128, 512], F32R, tag=f"wqk{k}", name=f"wqk{k}") for k in range(N_DIMB)]
        wv = [pp.tile([128, 256], F32R, tag=f"wv{k}", name=f"wv{k}") for k in range(N_DIMB)]
        wo = [pp.tile([128, DIM], F32R, tag=f"wo{d}", name=f"wo{d}") for d in range(2)]
        # q/k head-dim-major: partition block hp holds heads (2hp, 2hp+1)
        qt = [
            [pp.tile([128, QT], F32R, tag=f"qt{i}_{c}", name=f"qt{i}_{c}") for c in range(N_QT)]
            for i in range(2)
        ]
        kt = [
            [pp.tile([128, QT], F32R, tag=f"kt{i}_{c}", name=f"kt{i}_{c}") for c in range(N_QT)]
            for i in range(2)
        ]
        # v (token-major) + ones column, per key block: [128, 4 heads, 65]
        vh = [
            pp.tile([128, H_PER_CORE, HD + 1], F32R, tag=f"vh{b}", name=f"vh{b}")
            for b in range(N_KB)
        ]
        # ones row for the K=1 denominator-broadcast matmuls (row 64 used)
        ones = pp.tile([128, 64], F32R, tag="ones", name="ones")
        nc.vector.memset(ones[:].bitcast(F32), 1.0)

        # ---- input DMAs: kb-major so the kb=0..7 chains fill in order; the
        # xT columns arrive chunk-by-chunk so tile 0's projections start early
        for kb in range(N_DIMB):
            nc.sync.dma_start(wqk[kb][:], wqkT[kb * 128 : (kb + 1) * 128, :])
            nc.sync.dma_start(wv[kb][:], wvT[kb * 128 : (kb + 1) * 128, :])
            nc.sync.dma_start(xt[kb][0][:], xT[kb * 128 : (kb + 1) * 128, 0:QT])
        for ct in range(1, N_QT):
            cs = slice(ct * QT, (ct + 1) * QT)
            for kb in range(N_DIMB):
                nc.sync.dma_start(xt[kb][ct][:], xT[kb * 128 : (kb + 1) * 128, cs])
        for db in range(2):
            nc.sync.dma_start(wo[db][:], woT[db * 128 : (db + 1) * 128, :])

        def qk_chain(tt, ob):  # ob 0,1 -> q pair blocks; 2,3 -> k pair blocks
            ps = mmps.tile([128, 512], F32, tag="mm512", name=f"qk_ps{tt}_{ob}")
            for kb in range(N_DIMB):
                nc.tensor.matmul(
                    ps[:],
                    wqk[kb][:, ob * 128 : (ob + 1) * 128],
                    xt[kb][tt][:],
                    start=(kb == 0),
                    stop=(kb == N_DIMB - 1),
                )
            dest = (qt if ob < 2 else kt)[ob % 2][tt]
            nc.vector.tensor_copy(dest[:], ps[:])

        def v_chain(tb):
            ps = mmps.tile([128, 256], F32, tag="mm512", name=f"v_ps{tb}")
            for kb in range(N_DIMB):
                nc.tensor.matmul(
                    ps[:],
                    xt[kb][tb // 4][:, (tb % 4) * KB : (tb % 4 + 1) * KB],
                    wv[kb][:],
                    start=(kb == 0),
                    stop=(kb == N_DIMB - 1),
                )
            nc.vector.tensor_copy(vh[tb][:, :, 0:HD], ps[:])
            nc.vector.memset(vh[tb][:, :, HD : HD + 1].bitcast(F32), 1.0)

        def proj_pieces(tt):
            for ob in range(4):
                yield lambda ob=ob: qk_chain(tt, ob)
            for tb in range(4 * tt, 4 * tt + 4):
                yield lambda tb=tb: v_chain(tb)

        def y_pieces(tt, os_pair):
            for t4 in range(4):
                trows = slice(t4 * 128, (t4 + 1) * 128)
                ysb = ypool.tile([128, DIM], F32, tag="ysb", name=f"ysb{tt}_{t4}")
                for jb in range(2):

                    def piece(t4=t4, jb=jb, ysb=ysb, trows=trows):
                        yps = mmps.tile(
                            [128, 512], F32, tag="mm512", name=f"y_ps{tt}_{t4}_{jb}"
                        )
                        for db in range(2):
                            nc.tensor.matmul(
                                yps[:],
                                os_pair[db][:, trows],
                                wo[db][:, jb * 512 : (jb + 1) * 512],
                                start=(db == 0),
                                stop=(db == 1),
                            )
                        nc.vector.tensor_copy(ysb[:, jb * 512 : (jb + 1) * 512], yps[:])
                        if jb == 1:
                            nc.sync.dma_start(
                                y[tt * QT + t4 * 128 : tt * QT + (t4 + 1) * 128, :],
                                ysb[:],
                            )

                    yield piece

        def attend(tt, os_pair, fillers):
            nb = 4 * (tt + 1)  # allowed key blocks for this query tile
            n_steps = 2 * nb
            step = 0
            done_fill = 0
            n_fill = len(fillers)

            def fill():
                nonlocal done_fill
                want = (step + 1) * n_fill // n_steps
                while done_fill < want and fillers:
                    fillers.popleft()()
                    done_fill += 1

            for hp in range(2):  # head pair (2hp, 2hp+1)
                ot = [
                    otps.tile([HD + 1, QT], F32, tag="ot", name=f"ot{tt}_{hp}_{i}")
                    for i in range(2)
                ]

                def s_mm(b):
                    """S^T for key block b, both heads, into one 2-bank tile."""
                    diag = b - 4 * tt
                    d = diag * 128 if diag >= 0 else 0
                    s = sps.tile([128, 2 * QT], F32, tag="s2", name=f"s{tt}_{hp}_{b}")
                    for i in range(2):
                        rows = slice(i * 64, i * 64 + 64)
                        nc.tensor.matmul(
                            s[:, i * QT + d : (i + 1) * QT],
                            kt[hp][b // 4][rows, (b % 4) * KB : (b % 4 + 1) * KB],
                            qt[hp][tt][rows, d:QT],
                            start=True,
                            stop=True,
                        )
                    return s

                s_tiles = {0: s_mm(0)}
                for b in range(nb):
                    if b + 1 < nb:
                        s_tiles[b + 1] = s_mm(b + 1)
                    diag = b - 4 * tt
                    d = diag * 128 if diag >= 0 else 0
                    s = s_tiles.pop(b)
                    p = ppool.tile([128, 2 * QT], F32R, tag="p", name=f"p{tt}_{hp}_{b}")
                    if diag < 0:
                        nc.scalar.activation(p[:], s[:], EXP, scale=SCALE)
                    else:
                        # per head: rows 0-63 attend cols >= d, rows 64-127
                        # cols >= d+64 (64-chunk block-causal inside the tile)
                        s2 = s[:].rearrange("p (h c) -> p h c", h=2)
                        p2 = p[:].rearrange("p (h c) -> p h c", h=2)
                        nc.vector.memset(p2[64:128, :, d : d + 64].bitcast(F32), 0.0)
                        nc.scalar.activation(
                            p2[0:64, :, d:QT], s2[0:64, :, d:QT], EXP, scale=SCALE
                        )
                        if d + 64 < QT:
                            nc.scalar.activation(
                                p2[64:128, :, d + 64 : QT],
                                s2[64:128, :, d + 64 : QT],
                                EXP,
                                scale=SCALE,
                            )
                    for i in range(2):
                        nc.tensor.matmul(
                            ot[i][:, d:QT],
                            vh[b][:, 2 * hp + i, :],
                            p[:, i * QT + d : (i + 1) * QT],
                            start=(b == 0),
                            stop=(b == nb - 1),
                        )
                    fill()
                    step += 1

                # normalize: os_pair[hp][i*64:(i+1)*64] = ot[i][0:64] / ot[i][64]
                for i in range(2):
                    dn = rpool.tile([65, QT], F32R, tag="dn", name=f"dn{tt}_{hp}_{i}")
                    nc.vector.tensor_copy(dn[64:65, :], ot[i][64:65, :])
                    # broadcast the denominator row across 64 partitions via a
                    # K=1 ones matmul (gpsimd partition_broadcast mis-reads
                    # partition-64 sources on HW)
                    rbp = mmps.tile(
                        [128, 512], F32, tag="mm512", name=f"rbp{tt}_{hp}_{i}"
                    )
                    nc.tensor.matmul(
                        rbp[0:64, :], ones[64:65, :], dn[64:65, :],
                        start=True, stop=True,
                    )
                    rb = rpool.tile([64, QT], F32, tag="rb", name=f"rb{tt}_{hp}_{i}")
                    nc.vector.reciprocal(rb[:], rbp[0:64, :])
                    sc = scpool.tile([64, QT], F32R, tag="sc", name=f"sc{tt}_{hp}_{i}")
                    nc.vector.tensor_mul(sc[:], ot[i][0:64, :], rb[:])
                    # partition-shifting SBUF->SBUF copy via DMA
                    nc.sync.dma_start(os_pair[hp][i * 64 : (i + 1) * 64, :], sc[:])

            while fillers:
                fillers.popleft()()

        # ---- the pipeline ----
        for piece in proj_pieces(0):
            piece()
        prev_y = None
        for tt in range(N_QT):
            os_pair = [
                ospool.tile([128, QT], F32R, tag=f"os{i}", name=f"os{i}_{tt}")
                for i in range(2)
            ]
            fillers = deque()
            a = deque(proj_pieces(tt + 1)) if tt + 1 < N_QT else deque()
            b = deque(prev_y) if prev_y is not None else deque()
            while a or b:
                if a:
                    fillers.append(a.popleft())
                if b:
                    fillers.append(b.popleft())
            attend(tt, os_pair, fillers)
            prev_y = list(y_pieces(tt, os_pair))
        for piece in prev_y:
            piece()


def build():
    global _CACHED_NC
    if _CACHED_NC is not None:
        return _CACHED_NC
    nc = bacc.Bacc(
        "TRN2", target_bir_lowering=False, debug=False, enable_asserts=False
    )
    xT = nc.dram_tensor("xT", [DIM, T], F32R, kind="ExternalInput").ap()
    wqkT = nc.dram_tensor("wqkT", [DIM, 512], F32R, kind="ExternalInput").ap()
    wvT = nc.dram_tensor("wvT", [DIM, 256], F32R, kind="ExternalInput").ap()
    woT = nc.dram_tensor("woutT", [256, DIM], F32R, kind="ExternalInput").ap()
    y = nc.dram_tensor("y", [T, DIM], F32, kind="ExternalOutput").ap()
    with tile.TileContext(nc) as tc:
        _emit(nc, tc, xT, wqkT, wvT, woT, y)
    nc.compile()
    _CACHED_NC = nc
    return nc


def make_in_maps(x, Wqkv, Wout):
    """Host-side sharding: core c = (batch c//4, head-group c%4)."""
    in_maps = []
    for c in range(8):
        b, hg = divmod(c, 4)
        hs = hg * H_PER_CORE
        r0, r1 = hs * HD, (hs + H_PER_CORE) * HD
        qrows = Wqkv[r0:r1]
        krows = Wqkv[DIM + r0 : DIM + r1]
        vrows = Wqkv[2 * DIM + r0 : 2 * DIM + r1]
        in_maps.append(
            {
                "xT": np.ascontiguousarray(x[b].T),
                "wqkT": np.ascontiguousarray(np.concatenate([qrows, krows], 0).T),
                "wvT": np.ascontiguousarray(vrows.T),
                "woutT": np.ascontiguousarray(Wout[:, r0:r1].T),
            }
        )
    return in_maps


def kernel(x, Wqkv, Wout):
    x = np.asarray(x, dtype=np.float32)
    Wqkv = np.asarray(Wqkv, dtype=np.float32)
    Wout = np.asarray(Wout, dtype=np.float32)
    nc = build()
    in_maps = make_in_maps(x, Wqkv, Wout)
    res = run_bass_kernel_spmd(nc, in_maps, core_ids=list(range(8)))
    out = np.zeros((B, T, DIM), np.float32)
    for c in range(8):
        out[c // 4] += res.results[c]["y"]
    return out
